# revision 2
# baseline (speedup 1.0000x reference)
"""Trainium2 Bass kernel for channel-attention + 2 residual conv blocks.

Data-parallel over batch (8 cores, 1 batch element each). Two SPMD launches:
  A) accumulate S_big = [q|k]^T [q|k] over all 65536 pixels (channel Gram,
     includes q/k norms on diagonal blocks; conv biases folded via ones-row).
  B) folded attention-v projection (Wav = softmax-attn @ Wv) + 4 3x3 convs
     (9 shifted-view matmuls each, pitch-258 zero-gap row layout) + residuals.
Host does only the O(64^2) softmax/fold algebra between launches.
"""
import sys, os
for p in ('/opt/trn_rl_repo', os.path.expanduser('~/.axon_site/_ro/trn_rl_repo')):
    if os.path.isdir(p) and p not in sys.path:
        sys.path.insert(0, p)

import numpy as np
import ml_dtypes
import concourse.bass as bass
import concourse.bacc as bacc
import concourse.tile as tile
from concourse import mybir
from concourse.bass_utils import run_bass_kernel_spmd

dt = mybir.dt
F32, BF16 = dt.float32, dt.bfloat16
BF = ml_dtypes.bfloat16
AF = mybir.ActivationFunctionType
OP = mybir.AluOpType

D = 64
HW = 65536
H = W_IMG = 256
NCORES = 8
R = 32  # rows per block in pass B


def _build_nc_a():
    nc = bacc.Bacc("TRN2", target_bir_lowering=False, debug=False)
    xa = nc.dram_tensor("xa", [D + 1, HW], BF16, kind="ExternalInput").ap()
    ya = nc.dram_tensor("ya", [D + 1, HW], BF16, kind="ExternalInput").ap()
    wq = nc.dram_tensor("wq", [D + 1, D], BF16, kind="ExternalInput").ap()
    wk = nc.dram_tensor("wk", [D + 1, D], BF16, kind="ExternalInput").ap()
    sbig = nc.dram_tensor("sbig", [128, 128], F32, kind="ExternalOutput").ap()

    CH = 4096          # pixels per DMA chunk
    NCH = HW // CH     # 16
    NIT = CH // 128    # 32 inner steps

    with tile.TileContext(nc) as tc:
        with tc.tile_pool(name="consts", bufs=1) as consts, \
             tc.tile_pool(name="io", bufs=3) as io, \
             tc.tile_pool(name="work", bufs=4) as work, \
             tc.tile_pool(name="qkps", bufs=4, space="PSUM") as qkps, \
             tc.tile_pool(name="accps", bufs=1, space="PSUM") as accps:
            wqt = consts.tile([D + 1, D], BF16)
            wkt = consts.tile([D + 1, D], BF16)
            nc.sync.dma_start(out=wqt, in_=wq)
            nc.sync.dma_start(out=wkt, in_=wk)
            sb = accps.tile([128, 128], F32)
            it = 0
            for c in range(NCH):
                xt = io.tile([D + 1, CH], BF16, tag="xt")
                yt = io.tile([D + 1, CH], BF16, tag="yt")
                nc.sync.dma_start(out=xt, in_=xa[:, c * CH:(c + 1) * CH])
                nc.sync.dma_start(out=yt, in_=ya[:, c * CH:(c + 1) * CH])
                for j in range(NIT):
                    qk_ps = qkps.tile([128, 128], F32)
                    nc.tensor.matmul(qk_ps[:, 0:64], xt[:, j * 128:(j + 1) * 128],
                                     wqt, start=True, stop=True)
                    nc.tensor.matmul(qk_ps[:, 64:128], yt[:, j * 128:(j + 1) * 128],
                                     wkt, start=True, stop=True)
                    qk = work.tile([128, 128], BF16)
                    nc.scalar.activation(out=qk[:, 0:64], in_=qk_ps[:, 0:64],
                                         func=AF.Copy, bias=0.0, scale=1.0)
                    nc.vector.tensor_copy(out=qk[:, 64:128], in_=qk_ps[:, 64:128])
                    nc.tensor.matmul(sb, qk, qk, start=(it == 0), stop=(it == NCH * NIT - 1),
                                     skip_group_check=True)
                    it += 1
            sout = work.tile([128, 128], F32, tag="sout")
            nc.vector.tensor_copy(out=sout, in_=sb)
            nc.sync.dma_start(out=sbig, in_=sout)
    nc.compile()
    return nc


def _build_nc_b():
    nc = bacc.Bacc("TRN2", target_bir_lowering=False, debug=False)
    yb = nc.dram_tensor("yb", [D, HW], BF16, kind="ExternalInput").ap()
    yf = nc.dram_tensor("yf", [D, HW], F32, kind="ExternalInput").ap()
    wavt = nc.dram_tensor("wavt", [D, D], BF16, kind="ExternalInput").ap()
    bav = nc.dram_tensor("bav", [D, 1], F32, kind="ExternalInput").ap()
    wt_d = [nc.dram_tensor(f"w{i}", [D, 9 * D], BF16, kind="ExternalInput").ap()
            for i in range(1, 5)]
    bt_d = [nc.dram_tensor(f"b{i}", [D, 1], F32, kind="ExternalInput").ap()
            for i in range(1, 5)]
    out = nc.dram_tensor("out", [D, HW], F32, kind="ExternalOutput").ap()

    def extend_even(lo, hi):
        lo, hi = max(0, lo), min(256, hi)
        if (hi - lo) % 2:
            if hi < 256:
                hi += 1
            else:
                lo -= 1
        return lo, hi

    with tile.TileContext(nc) as tc:
        with tile_pools(tc) as (consts, ybp, stage, small, ps):
            wavt_t = consts.tile([D, D], BF16)
            bav_t = consts.tile([D, 1], F32)
            nc.sync.dma_start(out=wavt_t, in_=wavt)
            nc.sync.dma_start(out=bav_t, in_=bav)
            wts, bts = [], []
            for i in range(4):
                w = consts.tile([D, 9 * D], BF16, tag=f"w{i}")
                b = consts.tile([D, 1], F32, tag=f"b{i}")
                nc.sync.dma_start(out=w, in_=wt_d[i])
                nc.sync.dma_start(out=b, in_=bt_d[i])
                wts.append(w)
                bts.append(b)

            for blk in range(256 // R):
                r0, r1 = blk * R, (blk + 1) * R
                c3_lo, c3_hi = extend_even(r0 - 1, r1 + 1)
                o3_lo, o3_hi = extend_even(c3_lo - 1, c3_hi + 1)
                c1_lo, c1_hi = extend_even(o3_lo - 1, o3_hi + 1)
                t0_lo, t0_hi = extend_even(c1_lo - 1, c1_hi + 1)

                def mkstage(tag, lo, hi):
                    vlo = 1 if lo == 0 else 0
                    vhi = 1 if hi == 256 else 0
                    buf = stage.tile([D, 41, 258], BF16, tag=tag)
                    # zero-gap columns for dx-shift reads
                    nc.vector.memset(buf[:, :, 0:1], 0.0)
                    nc.vector.memset(buf[:, :, 257:258], 0.0)
                    if vlo:
                        nc.vector.memset(buf[:, 0, :], 0.0)
                    if vhi:
                        nc.vector.memset(buf[:, (hi - lo) + vlo, :], 0.0)
                    off = lo - vlo
                    return buf, off  # slot(row) = row - off

                t0, t0_off = mkstage("t0", t0_lo, t0_hi)
                c1, c1_off = mkstage("c1", c1_lo, c1_hi)
                o3, o3_off = mkstage("o3", o3_lo, o3_hi)
                c3, c3_off = mkstage("c3", c3_lo, c3_hi)

                ybt = ybp.tile([D, 40, 256], BF16, tag="ybt")
                nc.sync.dma_start(
                    out=ybt[:, 0:t0_hi - t0_lo, :],
                    in_=yb[:, t0_lo * 256:t0_hi * 256])

                # out2 = Wav @ y + bav  -> t0
                for pr in range(t0_lo, t0_hi, 2):
                    p = ps.tile([D, 512], F32)
                    nc.tensor.matmul(p, wavt_t, ybt[:, pr - t0_lo:pr - t0_lo + 2, :],
                                     start=True, stop=True)
                    nc.scalar.activation(out=t0[:, pr - t0_off:pr - t0_off + 2, 1:257],
                                         in_=p, func=AF.Identity, bias=bav_t, scale=1.0)

                def conv(src, src_off, dst_lo, dst_hi, wtile):
                    outs = []
                    for pr in range(dst_lo, dst_hi, 2):
                        p = ps.tile([D, 512], F32)
                        for t in range(9):
                            ky, kx = t // 3, t % 3
                            s = pr + ky - 1 - src_off
                            nc.tensor.matmul(p, wtile[:, t * 64:(t + 1) * 64],
                                             src[:, s:s + 2, kx:kx + 256],
                                             start=(t == 0), stop=(t == 8))
                        outs.append((pr, p))
                    return outs

                # conv1 + relu -> c1
                for pr, p in conv(t0, t0_off, c1_lo, c1_hi, wts[0]):
                    nc.scalar.activation(out=c1[:, pr - c1_off:pr - c1_off + 2, 1:257],
                                         in_=p, func=AF.Relu, bias=bts[0], scale=1.0)
                # conv2 + bias + residual t0 -> o3
                for pr, p in conv(c1, c1_off, o3_lo, o3_hi, wts[1]):
                    nc.vector.scalar_tensor_tensor(
                        out=o3[:, pr - o3_off:pr - o3_off + 2, 1:257],
                        in0=p, scalar=bts[1],
                        in1=t0[:, pr - t0_off:pr - t0_off + 2, 1:257],
                        op0=OP.add, op1=OP.add)
                # conv3 + relu -> c3
                for pr, p in conv(o3, o3_off, c3_lo, c3_hi, wts[2]):
                    nc.scalar.activation(out=c3[:, pr - c3_off:pr - c3_off + 2, 1:257],
                                         in_=p, func=AF.Relu, bias=bts[2], scale=1.0)
                # conv4 + bias + residual o3 + y -> out
                for pr, p in conv(c3, c3_off, r0, r1, wts[3]):
                    stg = small.tile([D, 512], F32, tag="stg")
                    nc.vector.scalar_tensor_tensor(
                        out=stg, in0=p, scalar=bts[3],
                        in1=o3[:, pr - o3_off:pr - o3_off + 2, 1:257],
                        op0=OP.add, op1=OP.add)
                    yft = small.tile([D, 512], F32, tag="yft")
                    nc.sync.dma_start(out=yft, in_=yf[:, pr * 256:(pr + 2) * 256])
                    stg2 = small.tile([D, 512], F32, tag="stg2")
                    nc.vector.tensor_tensor(out=stg2, in0=stg, in1=yft, op=OP.add)
                    nc.sync.dma_start(out=out[:, pr * 256:(pr + 2) * 256], in_=stg2)
    nc.compile()
    return nc


def tile_pools(tc):
    import contextlib

    @contextlib.contextmanager
    def cm():
        with tc.tile_pool(name="consts", bufs=1) as consts, \
             tc.tile_pool(name="ybp", bufs=2) as ybp, \
             tc.tile_pool(name="stage", bufs=1) as stage, \
             tc.tile_pool(name="small", bufs=4) as small, \
             tc.tile_pool(name="ps", bufs=6, space="PSUM") as ps:
            yield consts, ybp, stage, small, ps
    return cm()


_NC_CACHE = {}


def _get_ncs():
    if "a" not in _NC_CACHE:
        _NC_CACHE["a"] = _build_nc_a()
        _NC_CACHE["b"] = _build_nc_b()
    return _NC_CACHE["a"], _NC_CACHE["b"]


def _host_fold(sb, vw, vb):
    """S_big [128,128] -> (WavT bf16 [64,64], bav f32 [64,1])."""
    s = sb.astype(np.float64)
    S = s[0:64, 64:128]
    qss = np.diag(s[0:64, 0:64])
    kss = np.diag(s[64:128, 64:128])
    iq = 1.0 / np.maximum(np.sqrt(qss), 1e-12)
    ik = 1.0 / np.maximum(np.sqrt(kss), 1e-12)
    Ss = S * iq[:, None] * ik[None, :]
    A = np.zeros((64, 64), np.float64)
    for h in range(4):
        blk = Ss[16 * h:16 * h + 16, 16 * h:16 * h + 16]
        e = np.exp(blk - blk.max(axis=1, keepdims=True))
        A[16 * h:16 * h + 16, 16 * h:16 * h + 16] = e / e.sum(axis=1, keepdims=True)
    Wav = A @ vw.astype(np.float64)
    bav = A @ vb.astype(np.float64)
    return Wav.T.astype(np.float32).astype(BF), bav.astype(np.float32).reshape(64, 1)


def kernel(x, y, qw, qb, kw, kb, vw, vb,
           r1w1, r1b1, r1w2, r1b2, r2w1, r2b1, r2w2, r2b2, **_):
    x = np.asarray(x, np.float32)
    y = np.asarray(y, np.float32)
    nca, ncb = _get_ncs()

    ones = np.ones((1, HW), np.float32)
    wq_aug = np.concatenate([qw[:, :, 0, 0].T, qb[None, :]], axis=0).astype(BF)
    wk_aug = np.concatenate([kw[:, :, 0, 0].T, kb[None, :]], axis=0).astype(BF)

    in_maps_a = []
    for c in range(NCORES):
        xa = np.concatenate([x[c].reshape(D, HW), ones], axis=0).astype(BF)
        ya = np.concatenate([y[c].reshape(D, HW), ones], axis=0).astype(BF)
        in_maps_a.append({"xa": xa, "ya": ya, "wq": wq_aug, "wk": wk_aug})
    res_a = run_bass_kernel_spmd(nca, in_maps_a, core_ids=list(range(NCORES)))

    # host: softmax + fold attention into v-projection
    taps = {}
    for nm, wc in (("w1", r1w1), ("w2", r1w2), ("w3", r2w1), ("w4", r2w2)):
        taps[nm] = np.concatenate(
            [wc[:, :, ky, kx].T for ky in range(3) for kx in range(3)],
            axis=1).astype(BF)
    biases = {"b1": r1b1, "b2": r1b2, "b3": r2b1, "b4": r2b2}

    in_maps_b = []
    for c in range(NCORES):
        wavt, bav = _host_fold(res_a.results[c]["sbig"], vw[:, :, 0, 0], vb)
        m = {"yb": y[c].reshape(D, HW).astype(BF),
             "yf": np.ascontiguousarray(y[c].reshape(D, HW)),
             "wavt": wavt, "bav": bav}
        for nm, v in taps.items():
            m[nm] = v
        for nm, v in biases.items():
            m[nm] = np.ascontiguousarray(v.astype(np.float32).reshape(D, 1))
        in_maps_b.append(m)
    res_b = run_bass_kernel_spmd(ncb, in_maps_b, core_ids=list(range(NCORES)))

    return np.stack([res_b.results[c]["out"].reshape(D, H, W_IMG)
                     for c in range(NCORES)]).astype(np.float32)


if __name__ == "__main__":
    rng = np.random.default_rng(0)
    ins = {
        "x": rng.standard_normal((8, D, H, W_IMG)).astype(np.float32),
        "y": rng.standard_normal((8, D, H, W_IMG)).astype(np.float32),
        "qw": (rng.standard_normal((D, D, 1, 1)) / 8).astype(np.float32),
        "qb": (rng.standard_normal(D) / 8).astype(np.float32),
        "kw": (rng.standard_normal((D, D, 1, 1)) / 8).astype(np.float32),
        "kb": (rng.standard_normal(D) / 8).astype(np.float32),
        "vw": (rng.standard_normal((D, D, 1, 1)) / 8).astype(np.float32),
        "vb": (rng.standard_normal(D) / 8).astype(np.float32),
    }
    for i in (1, 2):
        for j in (1, 2):
            ins[f"r{i}w{j}"] = (rng.standard_normal((D, D, 3, 3)) / 24).astype(np.float32)
            ins[f"r{i}b{j}"] = (rng.standard_normal(D) / 24).astype(np.float32)
    o = kernel(**ins)
    print("kernel ran, out shape", o.shape, "std", o.std())


# revision 3
# speedup vs baseline: 1.0344x; 1.0344x over previous
"""Trainium2 Bass kernel for channel-attention + 2 residual conv blocks.

Data-parallel over batch (8 cores, 1 batch element each). Two SPMD launches:
  A) accumulate S_big = [q|k]^T [q|k] over all 65536 pixels (channel Gram,
     includes q/k norms on diagonal blocks; conv biases folded via ones-row).
  B) folded attention-v projection (Wav = softmax-attn @ Wv) + 4 3x3 convs
     (9 shifted-view matmuls each, pitch-258 zero-gap row layout) + residuals.
Host does only the O(64^2) softmax/fold algebra between launches.
"""
import sys, os
for p in ('/opt/trn_rl_repo', os.path.expanduser('~/.axon_site/_ro/trn_rl_repo')):
    if os.path.isdir(p) and p not in sys.path:
        sys.path.insert(0, p)

import numpy as np
import ml_dtypes
import concourse.bass as bass
import concourse.bacc as bacc
import concourse.tile as tile
from concourse import mybir
from concourse.bass_utils import run_bass_kernel_spmd

dt = mybir.dt
F32, BF16 = dt.float32, dt.bfloat16
BF = ml_dtypes.bfloat16
AF = mybir.ActivationFunctionType
OP = mybir.AluOpType

D = 64
HW = 65536
H = W_IMG = 256
NCORES = 8
R = 32  # rows per block in pass B


def _build_nc_a():
    nc = bacc.Bacc("TRN2", target_bir_lowering=False, debug=False)
    xa = nc.dram_tensor("xa", [D + 1, HW], BF16, kind="ExternalInput").ap()
    ya = nc.dram_tensor("ya", [D + 1, HW], BF16, kind="ExternalInput").ap()
    wq = nc.dram_tensor("wq", [D + 1, D], BF16, kind="ExternalInput").ap()
    wk = nc.dram_tensor("wk", [D + 1, D], BF16, kind="ExternalInput").ap()
    sbig = nc.dram_tensor("sbig", [128, 128], F32, kind="ExternalOutput").ap()

    CH = 4096          # pixels per DMA chunk
    NCH = HW // CH     # 16
    NIT = CH // 128    # 32 inner steps

    with tile.TileContext(nc) as tc:
        with tc.tile_pool(name="consts", bufs=1) as consts, \
             tc.tile_pool(name="io", bufs=3) as io, \
             tc.tile_pool(name="work", bufs=4) as work, \
             tc.tile_pool(name="qkps", bufs=4, space="PSUM") as qkps, \
             tc.tile_pool(name="accps", bufs=1, space="PSUM") as accps:
            wqt = consts.tile([D + 1, D], BF16)
            wkt = consts.tile([D + 1, D], BF16)
            nc.sync.dma_start(out=wqt, in_=wq)
            nc.sync.dma_start(out=wkt, in_=wk)
            sb = accps.tile([128, 128], F32)
            it = 0
            for c in range(NCH):
                xt = io.tile([D + 1, CH], BF16, tag="xt")
                yt = io.tile([D + 1, CH], BF16, tag="yt")
                nc.sync.dma_start(out=xt, in_=xa[:, c * CH:(c + 1) * CH])
                nc.sync.dma_start(out=yt, in_=ya[:, c * CH:(c + 1) * CH])
                for j in range(NIT):
                    qk_ps = qkps.tile([128, 128], F32)
                    nc.tensor.matmul(qk_ps[:, 0:64], xt[:, j * 128:(j + 1) * 128],
                                     wqt, start=True, stop=True)
                    nc.tensor.matmul(qk_ps[:, 64:128], yt[:, j * 128:(j + 1) * 128],
                                     wkt, start=True, stop=True)
                    qk = work.tile([128, 128], BF16)
                    nc.scalar.activation(out=qk[:, 0:64], in_=qk_ps[:, 0:64],
                                         func=AF.Copy, bias=0.0, scale=1.0)
                    nc.vector.tensor_copy(out=qk[:, 64:128], in_=qk_ps[:, 64:128])
                    nc.tensor.matmul(sb, qk, qk, start=(it == 0), stop=(it == NCH * NIT - 1),
                                     skip_group_check=True)
                    it += 1
            sout = work.tile([128, 128], F32, tag="sout")
            nc.vector.tensor_copy(out=sout, in_=sb)
            nc.sync.dma_start(out=sbig, in_=sout)
    nc.compile()
    return nc


def _build_nc_b():
    nc = bacc.Bacc("TRN2", target_bir_lowering=False, debug=False)
    yb = nc.dram_tensor("yb", [D, HW], BF16, kind="ExternalInput").ap()
    yf = nc.dram_tensor("yf", [D, HW], F32, kind="ExternalInput").ap()
    wavt = nc.dram_tensor("wavt", [D, D], BF16, kind="ExternalInput").ap()
    bav = nc.dram_tensor("bav", [D, 1], F32, kind="ExternalInput").ap()
    wt_d = [(nc.dram_tensor(f"wp{i}", [2 * D, 3 * D], BF16, kind="ExternalInput").ap(),
             nc.dram_tensor(f"w2{i}", [2 * D, 3 * D], BF16, kind="ExternalInput").ap())
            for i in range(1, 5)]
    bt_d = [nc.dram_tensor(f"b{i}", [D, 1], F32, kind="ExternalInput").ap()
            for i in range(1, 5)]
    out = nc.dram_tensor("out", [D, HW], F32, kind="ExternalOutput").ap()

    def extend_even(lo, hi):
        lo, hi = max(0, lo), min(256, hi)
        if (hi - lo) % 2:
            if hi < 256:
                hi += 1
            else:
                lo -= 1
        return lo, hi

    with tile.TileContext(nc) as tc:
        with tile_pools(tc) as (consts, ybp, stage, small, ps):
            wavt_t = consts.tile([D, D], BF16)
            bav_t = consts.tile([D, 1], F32)
            nc.sync.dma_start(out=wavt_t, in_=wavt)
            nc.sync.dma_start(out=bav_t, in_=bav)
            wts, bts = [], []
            for i in range(4):
                wp = consts.tile([2 * D, 3 * D], BF16, tag=f"wp{i}")
                w2 = consts.tile([2 * D, 3 * D], BF16, tag=f"w2{i}")
                b = consts.tile([D, 1], F32, tag=f"b{i}")
                nc.sync.dma_start(out=wp, in_=wt_d[i][0])
                nc.sync.dma_start(out=w2, in_=wt_d[i][1])
                nc.sync.dma_start(out=b, in_=bt_d[i])
                wts.append((wp, w2))
                bts.append(b)

            for blk in range(256 // R):
                r0, r1 = blk * R, (blk + 1) * R
                c3_lo, c3_hi = extend_even(r0 - 1, r1 + 1)
                o3_lo, o3_hi = extend_even(c3_lo - 1, c3_hi + 1)
                c1_lo, c1_hi = extend_even(o3_lo - 1, o3_hi + 1)
                t0_lo, t0_hi = extend_even(c1_lo - 1, c1_hi + 1)

                def mkstage(tag, lo, hi):
                    vlo = 1 if lo == 0 else 0
                    vhi = 1 if hi == 256 else 0
                    L = (hi - lo) + vlo + vhi  # slots [0, L): real+virtual
                    buf = stage.tile([2 * D, 42, 258], BF16, tag=tag)
                    # zero-gap columns for dx-shift reads (both halves)
                    nc.vector.memset(buf[:, :, 0:1], 0.0)
                    nc.vector.memset(buf[:, :, 257:258], 0.0)
                    if vlo:
                        nc.vector.memset(buf[:, 0, :], 0.0)
                    if vhi:
                        nc.vector.memset(buf[:, L - 1, :], 0.0)
                    # top half (shifted dup) tail: slots [L-2, 42) never written
                    # by pair-dups; zero them so zero-weight K=128 taps stay finite
                    tail = max(L - 2, 0)
                    nc.vector.memset(buf[D:2 * D, tail:42, :], 0.0)
                    off = lo - vlo
                    return buf, off  # slot(row) = row - off

                t0, t0_off = mkstage("t0", t0_lo, t0_hi)
                c1, c1_off = mkstage("c1", c1_lo, c1_hi)
                o3, o3_off = mkstage("o3", o3_lo, o3_hi)
                c3, c3_off = mkstage("c3", c3_lo, c3_hi)

                ybt = ybp.tile([D, 40, 256], BF16, tag="ybt")
                nc.sync.dma_start(
                    out=ybt[:, 0:t0_hi - t0_lo, :],
                    in_=yb[:, t0_lo * 256:t0_hi * 256])

                def dup(buf, q):
                    # top half slot t mirrors bottom slot t+1
                    a = max(q - 1, 0)
                    nc.gpsimd.tensor_copy(out=buf[D:2 * D, a:q + 1, :],
                                          in_=buf[0:D, a + 1:q + 2, :])

                # out2 = Wav @ y + bav  -> t0
                for pr in range(t0_lo, t0_hi, 2):
                    p = ps.tile([D, 512], F32)
                    nc.tensor.matmul(p, wavt_t, ybt[:, pr - t0_lo:pr - t0_lo + 2, :],
                                     start=True, stop=True)
                    nc.scalar.activation(out=t0[0:D, pr - t0_off:pr - t0_off + 2, 1:257],
                                         in_=p, func=AF.Identity, bias=bav_t, scale=1.0)
                    dup(t0, pr - t0_off)

                def conv(src, src_off, dst_lo, dst_hi, wtile):
                    wp, w2 = wtile
                    outs = []
                    for pr in range(dst_lo, dst_hi, 2):
                        p = ps.tile([D, 512], F32)
                        sl = pr - src_off
                        for kx in range(3):
                            # bottom: ky=0 rows (pr-1, pr); top: ky=1 rows (pr, pr+1)
                            nc.tensor.matmul(p, wp[:, kx * 64:(kx + 1) * 64],
                                             src[:, sl - 1:sl + 1, kx:kx + 256],
                                             start=(kx == 0), stop=False)
                        for kx in range(3):
                            # bottom: ky=2 rows (pr+1, pr+2); top: zero weights
                            nc.tensor.matmul(p, w2[:, kx * 64:(kx + 1) * 64],
                                             src[:, sl + 1:sl + 3, kx:kx + 256],
                                             start=False, stop=(kx == 2))
                        outs.append((pr, p))
                    return outs

                # conv1 + relu -> c1
                for pr, p in conv(t0, t0_off, c1_lo, c1_hi, wts[0]):
                    nc.scalar.activation(out=c1[0:D, pr - c1_off:pr - c1_off + 2, 1:257],
                                         in_=p, func=AF.Relu, bias=bts[0], scale=1.0)
                    dup(c1, pr - c1_off)
                # conv2 + bias + residual t0 -> o3
                for pr, p in conv(c1, c1_off, o3_lo, o3_hi, wts[1]):
                    nc.vector.scalar_tensor_tensor(
                        out=o3[0:D, pr - o3_off:pr - o3_off + 2, 1:257],
                        in0=p, scalar=bts[1],
                        in1=t0[0:D, pr - t0_off:pr - t0_off + 2, 1:257],
                        op0=OP.add, op1=OP.add)
                    dup(o3, pr - o3_off)
                # conv3 + relu -> c3
                for pr, p in conv(o3, o3_off, c3_lo, c3_hi, wts[2]):
                    nc.scalar.activation(out=c3[0:D, pr - c3_off:pr - c3_off + 2, 1:257],
                                         in_=p, func=AF.Relu, bias=bts[2], scale=1.0)
                    dup(c3, pr - c3_off)
                # conv4 + bias + residual o3 + y -> out
                for pr, p in conv(c3, c3_off, r0, r1, wts[3]):
                    stg = small.tile([D, 512], F32, tag="stg")
                    nc.vector.scalar_tensor_tensor(
                        out=stg, in0=p, scalar=bts[3],
                        in1=o3[0:D, pr - o3_off:pr - o3_off + 2, 1:257],
                        op0=OP.add, op1=OP.add)
                    yft = small.tile([D, 512], F32, tag="yft")
                    nc.sync.dma_start(out=yft, in_=yf[:, pr * 256:(pr + 2) * 256])
                    stg2 = small.tile([D, 512], F32, tag="stg2")
                    nc.vector.tensor_tensor(out=stg2, in0=stg, in1=yft, op=OP.add)
                    nc.sync.dma_start(out=out[:, pr * 256:(pr + 2) * 256], in_=stg2)
    nc.compile()
    return nc


def tile_pools(tc):
    import contextlib

    @contextlib.contextmanager
    def cm():
        with tc.tile_pool(name="consts", bufs=1) as consts, \
             tc.tile_pool(name="ybp", bufs=2) as ybp, \
             tc.tile_pool(name="stage", bufs=1) as stage, \
             tc.tile_pool(name="small", bufs=4) as small, \
             tc.tile_pool(name="ps", bufs=6, space="PSUM") as ps:
            yield consts, ybp, stage, small, ps
    return cm()


_NC_CACHE = {}


def _get_ncs():
    if "a" not in _NC_CACHE:
        _NC_CACHE["a"] = _build_nc_a()
        _NC_CACHE["b"] = _build_nc_b()
    return _NC_CACHE["a"], _NC_CACHE["b"]


def _host_fold(sb, vw, vb):
    """S_big [128,128] -> (WavT bf16 [64,64], bav f32 [64,1])."""
    s = sb.astype(np.float64)
    S = s[0:64, 64:128]
    qss = np.diag(s[0:64, 0:64])
    kss = np.diag(s[64:128, 64:128])
    iq = 1.0 / np.maximum(np.sqrt(qss), 1e-12)
    ik = 1.0 / np.maximum(np.sqrt(kss), 1e-12)
    Ss = S * iq[:, None] * ik[None, :]
    A = np.zeros((64, 64), np.float64)
    for h in range(4):
        blk = Ss[16 * h:16 * h + 16, 16 * h:16 * h + 16]
        e = np.exp(blk - blk.max(axis=1, keepdims=True))
        A[16 * h:16 * h + 16, 16 * h:16 * h + 16] = e / e.sum(axis=1, keepdims=True)
    Wav = A @ vw.astype(np.float64)
    bav = A @ vb.astype(np.float64)
    return Wav.T.astype(np.float32).astype(BF), bav.astype(np.float32).reshape(64, 1)


def kernel(x, y, qw, qb, kw, kb, vw, vb,
           r1w1, r1b1, r1w2, r1b2, r2w1, r2b1, r2w2, r2b2, **_):
    x = np.asarray(x, np.float32)
    y = np.asarray(y, np.float32)
    nca, ncb = _get_ncs()

    ones = np.ones((1, HW), np.float32)
    wq_aug = np.concatenate([qw[:, :, 0, 0].T, qb[None, :]], axis=0).astype(BF)
    wk_aug = np.concatenate([kw[:, :, 0, 0].T, kb[None, :]], axis=0).astype(BF)

    in_maps_a = []
    for c in range(NCORES):
        xa = np.concatenate([x[c].reshape(D, HW), ones], axis=0).astype(BF)
        ya = np.concatenate([y[c].reshape(D, HW), ones], axis=0).astype(BF)
        in_maps_a.append({"xa": xa, "ya": ya, "wq": wq_aug, "wk": wk_aug})
    res_a = run_bass_kernel_spmd(nca, in_maps_a, core_ids=list(range(NCORES)))

    # host: softmax + fold attention into v-projection
    taps = {}
    for i, wc in ((1, r1w1), (2, r1w2), (3, r2w1), (4, r2w2)):
        wp = np.concatenate(
            [np.concatenate([wc[:, :, 0, kx].T, wc[:, :, 1, kx].T], axis=0)
             for kx in range(3)], axis=1)
        w2 = np.concatenate(
            [np.concatenate([wc[:, :, 2, kx].T, np.zeros((D, D), np.float32)], axis=0)
             for kx in range(3)], axis=1)
        taps[f"wp{i}"] = wp.astype(BF)
        taps[f"w2{i}"] = w2.astype(BF)
    biases = {"b1": r1b1, "b2": r1b2, "b3": r2b1, "b4": r2b2}

    in_maps_b = []
    for c in range(NCORES):
        wavt, bav = _host_fold(res_a.results[c]["sbig"], vw[:, :, 0, 0], vb)
        m = {"yb": y[c].reshape(D, HW).astype(BF),
             "yf": np.ascontiguousarray(y[c].reshape(D, HW)),
             "wavt": wavt, "bav": bav}
        for nm, v in taps.items():
            m[nm] = v
        for nm, v in biases.items():
            m[nm] = np.ascontiguousarray(v.astype(np.float32).reshape(D, 1))
        in_maps_b.append(m)
    res_b = run_bass_kernel_spmd(ncb, in_maps_b, core_ids=list(range(NCORES)))

    return np.stack([res_b.results[c]["out"].reshape(D, H, W_IMG)
                     for c in range(NCORES)]).astype(np.float32)


if __name__ == "__main__":
    rng = np.random.default_rng(0)
    ins = {
        "x": rng.standard_normal((8, D, H, W_IMG)).astype(np.float32),
        "y": rng.standard_normal((8, D, H, W_IMG)).astype(np.float32),
        "qw": (rng.standard_normal((D, D, 1, 1)) / 8).astype(np.float32),
        "qb": (rng.standard_normal(D) / 8).astype(np.float32),
        "kw": (rng.standard_normal((D, D, 1, 1)) / 8).astype(np.float32),
        "kb": (rng.standard_normal(D) / 8).astype(np.float32),
        "vw": (rng.standard_normal((D, D, 1, 1)) / 8).astype(np.float32),
        "vb": (rng.standard_normal(D) / 8).astype(np.float32),
    }
    for i in (1, 2):
        for j in (1, 2):
            ins[f"r{i}w{j}"] = (rng.standard_normal((D, D, 3, 3)) / 24).astype(np.float32)
            ins[f"r{i}b{j}"] = (rng.standard_normal(D) / 24).astype(np.float32)
    o = kernel(**ins)
    print("kernel ran, out shape", o.shape, "std", o.std())


# revision 4
# speedup vs baseline: 1.0553x; 1.0202x over previous
"""Trainium2 Bass kernel for channel-attention + 2 residual conv blocks.

Data-parallel over batch (8 cores, 1 batch element each). Two SPMD launches:
  A) accumulate S_big = [q|k]^T [q|k] over all 65536 pixels (channel Gram,
     includes q/k norms on diagonal blocks; conv biases folded via ones-row).
  B) folded attention-v projection (Wav = softmax-attn @ Wv) + 4 3x3 convs
     (9 shifted-view matmuls each, pitch-258 zero-gap row layout) + residuals.
Host does only the O(64^2) softmax/fold algebra between launches.
"""
import sys, os
for p in ('/opt/trn_rl_repo', os.path.expanduser('~/.axon_site/_ro/trn_rl_repo')):
    if os.path.isdir(p) and p not in sys.path:
        sys.path.insert(0, p)

import numpy as np
import ml_dtypes
import concourse.bass as bass
import concourse.bacc as bacc
import concourse.tile as tile
from concourse import mybir
from concourse.bass_utils import run_bass_kernel_spmd

dt = mybir.dt
F32, BF16 = dt.float32, dt.bfloat16
BF = ml_dtypes.bfloat16
AF = mybir.ActivationFunctionType
OP = mybir.AluOpType

D = 64
HW = 65536
H = W_IMG = 256
NCORES = 8
R = 32  # rows per block in pass B


def _build_nc_a():
    nc = bacc.Bacc("TRN2", target_bir_lowering=False, debug=False)
    xa = nc.dram_tensor("xa", [D + 1, HW], BF16, kind="ExternalInput").ap()
    ya = nc.dram_tensor("ya", [D + 1, HW], BF16, kind="ExternalInput").ap()
    wq = nc.dram_tensor("wq", [D + 1, D], BF16, kind="ExternalInput").ap()
    wk = nc.dram_tensor("wk", [D + 1, D], BF16, kind="ExternalInput").ap()
    sbig = nc.dram_tensor("sbig", [128, 128], F32, kind="ExternalOutput").ap()

    CH = 4096          # pixels per DMA chunk
    NCH = HW // CH     # 16
    NIT = CH // 128    # 32 inner steps

    with tile.TileContext(nc) as tc:
        with tc.tile_pool(name="consts", bufs=1) as consts, \
             tc.tile_pool(name="io", bufs=3) as io, \
             tc.tile_pool(name="work", bufs=4) as work, \
             tc.tile_pool(name="qkps", bufs=4, space="PSUM") as qkps, \
             tc.tile_pool(name="accps", bufs=1, space="PSUM") as accps:
            wqt = consts.tile([D + 1, D], BF16)
            wkt = consts.tile([D + 1, D], BF16)
            nc.sync.dma_start(out=wqt, in_=wq)
            nc.sync.dma_start(out=wkt, in_=wk)
            sb = accps.tile([128, 128], F32)
            it = 0
            for c in range(NCH):
                xt = io.tile([D + 1, CH], BF16, tag="xt")
                yt = io.tile([D + 1, CH], BF16, tag="yt")
                nc.sync.dma_start(out=xt, in_=xa[:, c * CH:(c + 1) * CH])
                nc.sync.dma_start(out=yt, in_=ya[:, c * CH:(c + 1) * CH])
                for j in range(NIT):
                    qk_ps = qkps.tile([128, 128], F32)
                    nc.tensor.matmul(qk_ps[:, 0:64], xt[:, j * 128:(j + 1) * 128],
                                     wqt, start=True, stop=True)
                    nc.tensor.matmul(qk_ps[:, 64:128], yt[:, j * 128:(j + 1) * 128],
                                     wkt, start=True, stop=True)
                    qk = work.tile([128, 128], BF16)
                    nc.scalar.activation(out=qk[:, 0:64], in_=qk_ps[:, 0:64],
                                         func=AF.Copy, bias=0.0, scale=1.0)
                    nc.vector.tensor_copy(out=qk[:, 64:128], in_=qk_ps[:, 64:128])
                    nc.tensor.matmul(sb, qk, qk, start=(it == 0), stop=(it == NCH * NIT - 1),
                                     skip_group_check=True)
                    it += 1
            sout = work.tile([128, 128], F32, tag="sout")
            nc.vector.tensor_copy(out=sout, in_=sb)
            nc.sync.dma_start(out=sbig, in_=sout)
    nc.compile()
    return nc


def _build_nc_b():
    nc = bacc.Bacc("TRN2", target_bir_lowering=False, debug=False)
    yb = nc.dram_tensor("yb", [D, HW], BF16, kind="ExternalInput").ap()
    yf = nc.dram_tensor("yf", [D, HW], F32, kind="ExternalInput").ap()
    wavt = nc.dram_tensor("wavt", [D, D], BF16, kind="ExternalInput").ap()
    bav = nc.dram_tensor("bav", [D, 1], F32, kind="ExternalInput").ap()
    wt_d = [(nc.dram_tensor(f"wp{i}", [2 * D, 3 * D], BF16, kind="ExternalInput").ap(),
             nc.dram_tensor(f"w2{i}", [2 * D, 3 * D], BF16, kind="ExternalInput").ap())
            for i in range(1, 5)]
    bt_d = [nc.dram_tensor(f"b{i}", [D, 1], F32, kind="ExternalInput").ap()
            for i in range(1, 5)]
    out = nc.dram_tensor("out", [D, HW], F32, kind="ExternalOutput").ap()

    def extend_even(lo, hi):
        lo, hi = max(0, lo), min(256, hi)
        if (hi - lo) % 2:
            if hi < 256:
                hi += 1
            else:
                lo -= 1
        return lo, hi

    with tile.TileContext(nc) as tc:
        with tile_pools(tc) as (consts, ybp, stage, small, ps):
            wavt_t = consts.tile([D, D], BF16)
            bav_t = consts.tile([D, 1], F32)
            nc.sync.dma_start(out=wavt_t, in_=wavt)
            nc.sync.dma_start(out=bav_t, in_=bav)
            wts, bts = [], []
            for i in range(4):
                wp = consts.tile([2 * D, 3 * D], BF16, tag=f"wp{i}")
                w2 = consts.tile([2 * D, 3 * D], BF16, tag=f"w2{i}")
                b = consts.tile([D, 1], F32, tag=f"b{i}")
                nc.sync.dma_start(out=wp, in_=wt_d[i][0])
                nc.sync.dma_start(out=w2, in_=wt_d[i][1])
                nc.sync.dma_start(out=b, in_=bt_d[i])
                wts.append((wp, w2))
                bts.append(b)

            for blk in range(256 // R):
                r0, r1 = blk * R, (blk + 1) * R
                c3_lo, c3_hi = extend_even(r0 - 1, r1 + 1)
                o3_lo, o3_hi = extend_even(c3_lo - 1, c3_hi + 1)
                c1_lo, c1_hi = extend_even(o3_lo - 1, o3_hi + 1)
                t0_lo, t0_hi = extend_even(c1_lo - 1, c1_hi + 1)

                def mkstage(tag, lo, hi):
                    vlo = 1 if lo == 0 else 0
                    vhi = 1 if hi == 256 else 0
                    L = (hi - lo) + vlo + vhi  # slots [0, L): real+virtual
                    buf = stage.tile([2 * D, 42, 258], BF16, tag=tag)
                    # zero-gap columns for dx-shift reads (both halves)
                    nc.vector.memset(buf[:, :, 0:1], 0.0)
                    nc.vector.memset(buf[:, :, 257:258], 0.0)
                    if vlo:
                        nc.vector.memset(buf[:, 0, :], 0.0)
                    if vhi:
                        nc.vector.memset(buf[:, L - 1, :], 0.0)
                    # top half (shifted dup) tail: slots [L-2, 42) never written
                    # by pair-dups; zero them so zero-weight K=128 taps stay finite
                    tail = max(L - 2, 0)
                    nc.vector.memset(buf[D:2 * D, tail:42, :], 0.0)
                    off = lo - vlo
                    return buf, off  # slot(row) = row - off

                t0, t0_off = mkstage("t0", t0_lo, t0_hi)
                c1, c1_off = mkstage("c1", c1_lo, c1_hi)
                o3, o3_off = mkstage("o3", o3_lo, o3_hi)
                c3, c3_off = mkstage("c3", c3_lo, c3_hi)

                ybt = ybp.tile([D, 40, 256], BF16, tag="ybt")
                nc.sync.dma_start(
                    out=ybt[:, 0:t0_hi - t0_lo, :],
                    in_=yb[:, t0_lo * 256:t0_hi * 256])

                def dup(buf, q):
                    # top half slot t mirrors bottom slot t+1
                    a = max(q - 1, 0)
                    nc.gpsimd.tensor_copy(out=buf[D:2 * D, a:q + 1, :],
                                          in_=buf[0:D, a + 1:q + 2, :])

                # out2 = Wav @ y + bav  -> t0
                for pr in range(t0_lo, t0_hi, 2):
                    p = ps.tile([D, 512], F32)
                    nc.tensor.matmul(p, wavt_t, ybt[:, pr - t0_lo:pr - t0_lo + 2, :],
                                     start=True, stop=True)
                    nc.scalar.activation(out=t0[0:D, pr - t0_off:pr - t0_off + 2, 1:257],
                                         in_=p, func=AF.Identity, bias=bav_t, scale=1.0)
                    dup(t0, pr - t0_off)

                def conv(src, src_off, dst_lo, dst_hi, wtile):
                    wp, w2 = wtile
                    outs = []
                    for pr in range(dst_lo, dst_hi, 2):
                        p = ps.tile([D, 512], F32)
                        sl = pr - src_off
                        for kx in range(3):
                            # bottom: ky=0 rows (pr-1, pr); top: ky=1 rows (pr, pr+1)
                            nc.tensor.matmul(p, wp[:, kx * 64:(kx + 1) * 64],
                                             src[:, sl - 1:sl + 1, kx:kx + 256],
                                             start=(kx == 0), stop=False)
                        for kx in range(3):
                            # bottom: ky=2 rows (pr+1, pr+2); top: zero weights
                            nc.tensor.matmul(p, w2[:, kx * 64:(kx + 1) * 64],
                                             src[:, sl + 1:sl + 3, kx:kx + 256],
                                             start=False, stop=(kx == 2))
                        outs.append((pr, p))
                    return outs

                # conv1 + relu -> c1
                for pr, p in conv(t0, t0_off, c1_lo, c1_hi, wts[0]):
                    nc.scalar.activation(out=c1[0:D, pr - c1_off:pr - c1_off + 2, 1:257],
                                         in_=p, func=AF.Relu, bias=bts[0], scale=1.0)
                    dup(c1, pr - c1_off)
                # conv2 + bias + residual t0 -> o3
                for pr, p in conv(c1, c1_off, o3_lo, o3_hi, wts[1]):
                    nc.vector.scalar_tensor_tensor(
                        out=o3[0:D, pr - o3_off:pr - o3_off + 2, 1:257],
                        in0=p, scalar=bts[1],
                        in1=t0[0:D, pr - t0_off:pr - t0_off + 2, 1:257],
                        op0=OP.add, op1=OP.add)
                    dup(o3, pr - o3_off)
                # conv3 + relu -> c3
                for pr, p in conv(o3, o3_off, c3_lo, c3_hi, wts[2]):
                    nc.scalar.activation(out=c3[0:D, pr - c3_off:pr - c3_off + 2, 1:257],
                                         in_=p, func=AF.Relu, bias=bts[2], scale=1.0)
                    dup(c3, pr - c3_off)
                # conv4 + bias + residual o3 + y -> out
                for pr, p in conv(c3, c3_off, r0, r1, wts[3]):
                    stg = small.tile([D, 512], F32, tag="stg")
                    nc.vector.scalar_tensor_tensor(
                        out=stg, in0=p, scalar=bts[3],
                        in1=o3[0:D, pr - o3_off:pr - o3_off + 2, 1:257],
                        op0=OP.add, op1=OP.add)
                    yft = small.tile([D, 512], F32, tag="yft")
                    nc.sync.dma_start(out=yft, in_=yf[:, pr * 256:(pr + 2) * 256])
                    stg2 = small.tile([D, 512], F32, tag="stg2")
                    nc.vector.tensor_tensor(out=stg2, in0=stg, in1=yft, op=OP.add)
                    nc.sync.dma_start(out=out[:, pr * 256:(pr + 2) * 256], in_=stg2)
    nc.compile()
    return nc


def tile_pools(tc):
    import contextlib

    @contextlib.contextmanager
    def cm():
        with tc.tile_pool(name="consts", bufs=1) as consts, \
             tc.tile_pool(name="ybp", bufs=2) as ybp, \
             tc.tile_pool(name="stage", bufs=1) as stage, \
             tc.tile_pool(name="small", bufs=4) as small, \
             tc.tile_pool(name="ps", bufs=6, space="PSUM") as ps:
            yield consts, ybp, stage, small, ps
    return cm()


_NC_CACHE = {}


def _get_ncs():
    if "a" not in _NC_CACHE:
        _NC_CACHE["a"] = _build_nc_a()
        _NC_CACHE["b"] = _build_nc_b()
    return _NC_CACHE["a"], _NC_CACHE["b"]


def _host_fold(sb, vw, vb):
    """S_big [128,128] -> (WavT bf16 [64,64], bav f32 [64,1])."""
    s = sb.astype(np.float64)
    S = s[0:64, 64:128]
    qss = np.diag(s[0:64, 0:64])
    kss = np.diag(s[64:128, 64:128])
    iq = 1.0 / np.maximum(np.sqrt(qss), 1e-12)
    ik = 1.0 / np.maximum(np.sqrt(kss), 1e-12)
    Ss = S * iq[:, None] * ik[None, :]
    A = np.zeros((64, 64), np.float64)
    for h in range(4):
        blk = Ss[16 * h:16 * h + 16, 16 * h:16 * h + 16]
        e = np.exp(blk - blk.max(axis=1, keepdims=True))
        A[16 * h:16 * h + 16, 16 * h:16 * h + 16] = e / e.sum(axis=1, keepdims=True)
    Wav = A @ vw.astype(np.float64)
    bav = A @ vb.astype(np.float64)
    return Wav.T.astype(np.float32).astype(BF), bav.astype(np.float32).reshape(64, 1)


def kernel(x, y, qw, qb, kw, kb, vw, vb,
           r1w1, r1b1, r1w2, r1b2, r2w1, r2b1, r2w2, r2b2, **_):
    x = np.asarray(x, np.float32)
    y = np.asarray(y, np.float32)
    qw, qb, kw, kb = (np.asarray(a, np.float32) for a in (qw, qb, kw, kb))
    vw, vb = np.asarray(vw, np.float32), np.asarray(vb, np.float32)
    r1w1, r1b1, r1w2, r1b2 = (np.asarray(a, np.float32) for a in (r1w1, r1b1, r1w2, r1b2))
    r2w1, r2b1, r2w2, r2b2 = (np.asarray(a, np.float32) for a in (r2w1, r2b1, r2w2, r2b2))
    nca, ncb = _get_ncs()

    ones = np.ones((1, HW), np.float32)
    wq_aug = np.concatenate([qw[:, :, 0, 0].T, qb[None, :]], axis=0).astype(BF)
    wk_aug = np.concatenate([kw[:, :, 0, 0].T, kb[None, :]], axis=0).astype(BF)

    in_maps_a = []
    for c in range(NCORES):
        xa = np.concatenate([x[c].reshape(D, HW), ones], axis=0).astype(BF)
        ya = np.concatenate([y[c].reshape(D, HW), ones], axis=0).astype(BF)
        in_maps_a.append({"xa": xa, "ya": ya, "wq": wq_aug, "wk": wk_aug})
    res_a = run_bass_kernel_spmd(nca, in_maps_a, core_ids=list(range(NCORES)))

    # host: softmax + fold attention into v-projection
    taps = {}
    for i, wc in ((1, r1w1), (2, r1w2), (3, r2w1), (4, r2w2)):
        wp = np.concatenate(
            [np.concatenate([wc[:, :, 0, kx].T, wc[:, :, 1, kx].T], axis=0)
             for kx in range(3)], axis=1)
        w2 = np.concatenate(
            [np.concatenate([wc[:, :, 2, kx].T, np.zeros((D, D), np.float32)], axis=0)
             for kx in range(3)], axis=1)
        taps[f"wp{i}"] = wp.astype(BF)
        taps[f"w2{i}"] = w2.astype(BF)
    biases = {"b1": r1b1, "b2": r1b2, "b3": r2b1, "b4": r2b2}

    in_maps_b = []
    for c in range(NCORES):
        wavt, bav = _host_fold(res_a.results[c]["sbig"], vw[:, :, 0, 0], vb)
        m = {"yb": y[c].reshape(D, HW).astype(BF),
             "yf": np.ascontiguousarray(y[c].reshape(D, HW)),
             "wavt": wavt, "bav": bav}
        for nm, v in taps.items():
            m[nm] = v
        for nm, v in biases.items():
            m[nm] = np.ascontiguousarray(v.astype(np.float32).reshape(D, 1))
        in_maps_b.append(m)
    res_b = run_bass_kernel_spmd(ncb, in_maps_b, core_ids=list(range(NCORES)))

    return np.stack([res_b.results[c]["out"].reshape(D, H, W_IMG)
                     for c in range(NCORES)]).astype(np.float32)


if __name__ == "__main__":
    rng = np.random.default_rng(0)
    ins = {
        "x": rng.standard_normal((8, D, H, W_IMG)).astype(np.float32),
        "y": rng.standard_normal((8, D, H, W_IMG)).astype(np.float32),
        "qw": (rng.standard_normal((D, D, 1, 1)) / 8).astype(np.float32),
        "qb": (rng.standard_normal(D) / 8).astype(np.float32),
        "kw": (rng.standard_normal((D, D, 1, 1)) / 8).astype(np.float32),
        "kb": (rng.standard_normal(D) / 8).astype(np.float32),
        "vw": (rng.standard_normal((D, D, 1, 1)) / 8).astype(np.float32),
        "vb": (rng.standard_normal(D) / 8).astype(np.float32),
    }
    for i in (1, 2):
        for j in (1, 2):
            ins[f"r{i}w{j}"] = (rng.standard_normal((D, D, 3, 3)) / 24).astype(np.float32)
            ins[f"r{i}b{j}"] = (rng.standard_normal(D) / 24).astype(np.float32)
    o = kernel(**ins)
    print("kernel ran, out shape", o.shape, "std", o.std())
